# revision 44
# baseline (speedup 1.0000x reference)
"""BBox-aware BCE loss kernel for Trainium2 (8 NeuronCores, data parallel).

Math (reformulation of the reference):
  loss = softplus(pred) - pred*target = softplus((1-2t)*pred)  for t in {0,1}
  u = 1-2t in {+1 (t=0), -1 (t=1)}
  Su(i,j) = sum of u over the clipped 5x5 window, V = clipped window size
  edge pixel <=> window is mixed <=> Su != V*u_center  (all-same windows
  have Su = V*u_c exactly; mixed have |Su| < V).  The center subtraction
  is folded into the band matmul: the center tap uses A_cen = band -
  V[row]*I, so Su' = Su - V*u_c and edge <=> Su' != 0 -- a single
  one-sided exact integer test, no abs/threshold pass needed.
  result = (sum(loss) - 0.9*sum(loss*edge)) / N  (equals the reference in
  both branches of its global `cond`: no edges anywhere => sum=0).

Approximation: V uses the interior row-window width 5 everywhere, so the
4 image-border columns (true width 3 or 4) always classify as "edge".
For random targets they almost always are (error ~1e-6 relative); the
worst case (solid target) is 4/1024 * 0.9 = 0.35% relative -- far inside
the 2e-2 gate.  Image top/bottom ROWS are exact via per-row V in A_cen.

Device pipeline per core (4 samples, 9 overlapping 128-row tiles each so
the 5-tap column window never crosses a tile boundary):
  GpSimd:  casting DMAs f32 HBM -> bf16 SBUF for pred and target
  VectorE: u = 1-2t (4x); pair sums vb = u + shift(u) (2x, odd tiles
           only); s = p*u (2x); one fused (Su' != 0)*loss pass straight
           from PSUM per tile
  ScalarE: g = exp(s); loss = ln(g*1+1) + per-row loss-sum accumulation,
           each fused across a tile group
  TensorE: Su' per 512-half as A@vb(0) + A@vb(3) + A_cen@u(2) on 1-of-4
           tiles, or plain 5-tap shifted matmuls on the rest (trades idle
           TensorE time for the vb pass on the busier VectorE)
Loss sums ride the ln pass's accum_out (per-row, per exp/ln group); groups
pair only tiles with identical owned-row ranges so the host can mask them.
Host: float64 reduction of per-core partials (owned-row masks per tile).
"""

import sys

import numpy as np

sys.path.insert(0, "/opt/trn_rl_repo")

import ml_dtypes

B, H, W = 32, 1024, 1024
NCORES = 8
SPC = B // NCORES  # samples per core
ROWS = SPC * H
N_TOT = float(B * H * W)
WP = W + 4  # padded width for the 5-tap row window

# per-sample tiling: (input_row_start, input_rows, owned_lo, owned_hi)
TILES = [(0, 128, 0, 126)]
for t in range(1, 8):
    TILES.append((124 * t, 128, 2, 126))
TILES.append((992, 32, 2, 32))
NT = len(TILES)  # 9
NTILES = SPC * NT  # 36

BF16 = ml_dtypes.bfloat16


def _band() -> np.ndarray:
    a = np.zeros((128, 128), dtype=np.float32)
    for k in range(128):
        for m in range(128):
            if abs(k - m) <= 2:
                a[k, m] = 1.0
    return a


def _vh_of(tile_idx: int) -> np.ndarray:
    """clipped column-window size per in-tile row."""
    in0, p_in, _, _ = TILES[tile_idx]
    vh = np.full(128, 5.0, dtype=np.float32)
    for k in range(p_in):
        img = in0 + k
        vh[k] = min(img, 2) + min(H - 1 - img, 2) + 1
    return vh


def _owned(tile_idx: int) -> np.ndarray:
    _, _, o0, o1 = TILES[tile_idx]
    m = np.zeros(128, dtype=np.float32)
    m[o0:o1] = 1.0
    return m


def _statics() -> dict[str, np.ndarray]:
    s = {"a_full": _band().astype(BF16)}
    for nm, ti in (("top", 0), ("mid", 1), ("last", 8)):
        vh = _vh_of(ti)
        s[f"own_{nm}"] = _owned(ti).astype(BF16).reshape(128, 1)
        # center-tap matrix: band - 5*vh[row] on the diagonal (all values
        # small integers, exact in bf16)
        ac = _band()
        for k in range(128):
            ac[k, k] -= 5.0 * vh[k]
        s[f"a_cen_{nm}"] = ac.astype(BF16)
    return s


_CACHED = {}


def _split_multi_waits(nc, mybir):
    """This walrus's core_v3 codegen allows only one sem-wait per
    instruction; peel extra waits onto same-engine NOPs placed just before."""
    skip = (mybir.InstEventSemaphore,)
    k = 0
    for fn in nc.m.functions:
        for blk in fn.blocks:
            out = []
            for inst in blk.instructions:
                si = inst.sync_info
                if (si is not None and len(si.on_wait) > 1
                        and not isinstance(inst, skip)):
                    waits = list(si.on_wait)
                    for w in waits[:-1]:
                        k += 1
                        nop = mybir.InstNoOp(name=f"wsplit-{k}", ins=[], outs=[])
                        nop.engine = inst.engine
                        nop.sync_info = mybir.SyncInfo(on_wait=[w], on_update=[])
                        out.append(nop)
                    inst.sync_info = mybir.SyncInfo(
                        on_wait=[waits[-1]], on_update=list(si.on_update))
                out.append(inst)
            blk.instructions = out


def _build_nc():
    import concourse.bass as bass
    import concourse.mybir as mybir
    import concourse.tile as tile

    f32 = mybir.dt.float32
    bf16 = mybir.dt.bfloat16
    Act = mybir.ActivationFunctionType
    Alu = mybir.AluOpType

    nc = bass.Bass("TRN2", target_bir_lowering=False, debug=False,
                   num_devices=NCORES, num_swdge_queues=4)

    pred_d = nc.dram_tensor("pred", [ROWS, W], f32, kind="ExternalInput").ap()
    tgt_d = nc.dram_tensor("target", [ROWS, W], f32, kind="ExternalInput").ap()
    sd = {}
    statics = _statics()
    for nm, arr in statics.items():
        sd[nm] = nc.dram_tensor(nm, list(arr.shape), bf16,
                                kind="ExternalInput").ap()
    NG = 6 * SPC  # loss-sum accumulation groups (see PAIRS)
    oedge_d = nc.dram_tensor("out_edge", [128, NTILES + NG], f32,
                             kind="ExternalOutput").ap()

    with tile.TileContext(nc) as tc:
        with (
            tc.tile_pool(name="sing", bufs=1) as sing,
            tc.tile_pool(name="tb", bufs=10) as tb_pool,
            tc.tile_pool(name="pb", bufs=10) as pb_pool,
            tc.tile_pool(name="vb", bufs=6) as vb_pool,
            tc.tile_pool(name="s", bufs=4) as s_pool,
            tc.tile_pool(name="g", bufs=4) as g_pool,
            tc.tile_pool(name="loss", bufs=6) as loss_pool,
            tc.tile_pool(name="scr", bufs=6) as scr_pool,
            tc.tile_pool(name="psum", bufs=4, space="PSUM") as psum_pool,
        ):
            # ---- statics in SBUF ----
            sb = {}
            for nm, arr in statics.items():
                sb[nm] = sing.tile(list(arr.shape), bf16, tag=nm, name=nm)
                nc.sync.dma_start(out=sb[nm][:], in_=sd[nm][:])

            def per_tile(t):
                nm = "top" if t == 0 else ("last" if t == NT - 1 else "mid")
                return sb[f"own_{nm}"], sb[f"a_cen_{nm}"]

            stats = sing.tile([128, NTILES + NG], f32, tag="stats")
            nc.vector.memset(stats[:], 0.0)

            # padded ring buffers (pads zeroed once, never rewritten)
            u_bufs = [sing.tile([128, WP], bf16, tag=f"ub{i}",
                                name=f"ub{i}") for i in range(6)]
            for bb in u_bufs:
                nc.vector.memset(bb[:, 0:2], 0.0)
                nc.vector.memset(bb[:, W + 2:WP], 0.0)

            a_sb = sb["a_full"]
            pending = []  # (idx, p_in, sup, loss2, blk)

            def stage_b():
                # Edge-masked loss sums for a tile whose Su'/loss are 2+
                # tiles old, so no engine queue head ever stalls on a
                # fresh cross-engine dependency.  One fused STT does the
                # whole edge test: (Su' != 0)*loss straight from PSUM
                # (exact integer test), accumulated per partition.  Host
                # masks rows to owned per tile.
                bidx, p_in, sup, loss2, blk = pending.pop(0)
                lo = blk * W
                scr = scr_pool.tile([128, W], bf16)
                nc.vector.scalar_tensor_tensor(
                    out=scr[0:p_in], in0=sup[0:p_in],
                    scalar=0.0, in1=loss2[0:p_in, lo:lo + W],
                    op0=Alu.not_equal, op1=Alu.mult,
                    accum_out=stats[0:p_in, bidx:bidx + 1])
                if bidx == 17:
                    # first half of the stats early, off the tail
                    nc.sync.dma_start(out=oedge_d[:, 0:18],
                                      in_=stats[:, 0:18])

            # DVE/DMA work stays per-tile (fine-grained pipelining); only
            # exp/ln are fused per tile-PAIR through a shared s2 buffer,
            # halving ScalarE's fixed per-instruction cost.
            # the 32-row tile goes FIRST: its small DMAs land ~4x
            # sooner, so the pipeline's first compute chain starts while
            # tile 0's full-size DMA is still streaming
            PAIRS = [(8,), (0,), (1, 2), (3, 4), (5, 6), (7,)]
            idx = 0
            uidx = 0
            aidx = 0
            for smp in range(SPC):
                for gi, ts in enumerate(PAIRS):
                    nblk = len(ts)
                    P = max(TILES[t][1] for t in ts)
                    s2 = s_pool.tile([128, 2048], bf16)
                    pair = []
                    for b, t in enumerate(ts):
                        in0, p_in, o0, o1 = TILES[t]
                        r0 = smp * H + in0

                        # casting DMAs: f32 HBM -> bf16 SBUF
                        tb = tb_pool.tile([128, W], bf16)
                        nc.gpsimd.dma_start(out=tb[0:p_in],
                                            in_=tgt_d[r0:r0 + p_in, :])
                        pb = pb_pool.tile([128, W], bf16)
                        nc.gpsimd.dma_start(out=pb[0:p_in],
                                            in_=pred_d[r0:r0 + p_in, :])

                        # u = 1 - 2t into padded buffer center (4x mode)
                        ub = u_bufs[uidx % 6]
                        uidx += 1
                        nc.vector.tensor_scalar(
                            out=ub[0:p_in, 2:2 + W], in0=tb[0:p_in],
                            scalar1=-2.0, scalar2=1.0, op0=Alu.mult,
                            op1=Alu.add)

                        # pair sums: vb[j] = u[j] + u[j+1]  (2x mode).
                        # Only every other tile: the even tiles run plain
                        # 5-tap matmuls instead, trading idle TensorE
                        # time for DVE time (DVE is the pacer).
                        if aidx % 4 == 3:
                            vb = vb_pool.tile([128, WP], bf16)
                            nc.vector.tensor_add(out=vb[0:p_in, 0:W + 3],
                                                 in0=ub[0:p_in, 0:W + 3],
                                                 in1=ub[0:p_in, 1:W + 4])
                        else:
                            vb = None

                        # s = p*u (all bf16, 2x) into this pair's block
                        nc.vector.tensor_mul(
                            out=s2[0:p_in, b * W:b * W + W],
                            in0=pb[0:p_in],
                            in1=ub[0:p_in, 2:2 + W])
                        pair.append((t, b, ub, vb))
                        aidx += 1

                    # loss = ln(exp(s)+1), the whole group in one ACT
                    # pass; ln accumulates per-row loss sums (host masks
                    # rows to the group's shared owned range)
                    g2 = g_pool.tile([128, 2048], bf16)
                    nc.scalar.activation(out=g2[0:P, 0:nblk * W],
                                         in_=s2[0:P, 0:nblk * W],
                                         func=Act.Exp)
                    gcol = NTILES + smp * len(PAIRS) + gi
                    loss2 = loss_pool.tile([128, 2048], bf16)
                    nc.scalar.activation(out=loss2[0:P, 0:nblk * W],
                                         in_=g2[0:P, 0:nblk * W],
                                         func=Act.Ln, bias=1.0,
                                         accum_out=stats[0:P,
                                                         gcol:gcol + 1])

                    # Su': either A@vb(0) + A@vb(3) + A_cen@u(2) (3-tap,
                    # vb tiles) or plain 5-tap shifted matmuls on u
                    for t, b, ub, vb in pair:
                        p_in = TILES[t][1]
                        _, acen_sb = per_tile(t)
                        sup = psum_pool.tile([128, W], f32)
                        for h in (0, 512):
                            if vb is not None:
                                nc.tensor.matmul(
                                    sup[:, h:h + 512], a_sb[0:p_in, :],
                                    vb[0:p_in, h:h + 512],
                                    start=True, stop=False)
                                nc.tensor.matmul(
                                    sup[:, h:h + 512], a_sb[0:p_in, :],
                                    vb[0:p_in, h + 3:h + 515],
                                    start=False, stop=False)
                            else:
                                for dd, first in ((0, True), (1, False),
                                                  (3, False), (4, False)):
                                    nc.tensor.matmul(
                                        sup[:, h:h + 512], a_sb[0:p_in, :],
                                        ub[0:p_in, h + dd:h + dd + 512],
                                        start=first, stop=False)
                            nc.tensor.matmul(
                                sup[:, h:h + 512], acen_sb[0:p_in, :],
                                ub[0:p_in, h + 2:h + 514],
                                start=False, stop=True)
                        pending.append((idx, p_in, sup, loss2, b))
                        idx += 1
                    # keep a 2-tile skew mid-run for decoupling, but
                    # drain eagerly near the end so the final STTs
                    # overlap the last groups' exp/ln/matmul work
                    lim = 2 if (smp < SPC - 1 or gi < 4) else 0
                    while len(pending) > lim:
                        stage_b()
            while pending:
                stage_b()

            nc.sync.dma_start(out=oedge_d[:, 18:NTILES + NG],
                              in_=stats[:, 18:NTILES + NG])

    _split_multi_waits(nc, mybir)
    return nc


def _get_nc():
    if "nc" not in _CACHED:
        _CACHED["nc"] = _build_nc()
    return _CACHED["nc"]


def run(pred: np.ndarray, target: np.ndarray, trace: bool = False):
    """Returns (result_scalar, BassKernelResults)."""
    from concourse import bass_utils

    nc = _get_nc()
    statics = _statics()
    pred = np.ascontiguousarray(np.asarray(pred).reshape(B * H, W),
                                dtype=np.float32)
    target = np.ascontiguousarray(np.asarray(target).reshape(B * H, W),
                                  dtype=np.float32)
    in_maps = []
    for c in range(NCORES):
        m = dict(statics)
        m["pred"] = pred[c * ROWS:(c + 1) * ROWS]
        m["target"] = target[c * ROWS:(c + 1) * ROWS]
        in_maps.append(m)
    res = bass_utils.run_bass_kernel_spmd(
        nc, in_maps, core_ids=list(range(NCORES)), trace=trace)
    # loss-sum group -> shared owned-row range (matches PAIRS in build)
    GROUPS = [(8,), (0,), (1, 2), (3, 4), (5, 6), (7,)]
    s_loss = 0.0
    s_le = 0.0
    for r in res.results:
        e = r["out_edge"].astype(np.float64)
        order = [8, 0, 1, 2, 3, 4, 5, 6, 7]  # per-sample processing order
        for ti in range(NTILES):
            _, _, o0, o1 = TILES[order[ti % NT]]
            s_le += e[o0:o1, ti].sum()
        for smp in range(SPC):
            for gi, ts in enumerate(GROUPS):
                _, _, o0, o1 = TILES[ts[0]]
                gcol = NTILES + smp * len(GROUPS) + gi
                s_loss += e[o0:o1, gcol].sum()
    val = np.float32((s_loss - 0.9 * s_le) / N_TOT)
    return np.asarray(val, dtype=np.float32), res


def kernel(pred: np.ndarray, target: np.ndarray) -> np.ndarray:
    val, _ = run(pred, target, trace=False)
    return val


if __name__ == "__main__":
    rng = np.random.default_rng(0)
    p = rng.standard_normal((B, 1, H, W)).astype(np.float32)
    t = rng.integers(0, 2, (B, 1, H, W)).astype(np.float32)
    print(kernel(pred=p, target=t))


# revision 45
# speedup vs baseline: 1.1272x; 1.1272x over previous
"""BBox-aware BCE loss kernel for Trainium2 (8 NeuronCores, data parallel).

Math (reformulation of the reference):
  loss = softplus(pred) - pred*target = softplus((1-2t)*pred)  for t in {0,1}
  u = 1-2t in {+1 (t=0), -1 (t=1)}
  Su(i,j) = sum of u over the clipped 5x5 window, V = clipped window size
  edge pixel <=> window is mixed <=> Su != V*u_center  (all-same windows
  have Su = V*u_c exactly; mixed have |Su| < V).  The center subtraction
  is folded into the band matmul: the center tap uses A_cen = band -
  V[row]*I, so Su' = Su - V*u_c and edge <=> Su' != 0 -- a single
  one-sided exact integer test, no abs/threshold pass needed.
  result = (sum(loss) - 0.9*sum(loss*edge)) / N  (equals the reference in
  both branches of its global `cond`: no edges anywhere => sum=0).

Approximation: V uses the interior row-window width 5 everywhere, so the
4 image-border columns (true width 3 or 4) always classify as "edge".
For random targets they almost always are (error ~1e-6 relative); the
worst case (solid target) is 4/1024 * 0.9 = 0.35% relative -- far inside
the 2e-2 gate.  Image top/bottom ROWS are exact via per-row V in A_cen.

Device pipeline per core (4 samples, 9 overlapping 128-row tiles each so
the 5-tap column window never crosses a tile boundary):
  GpSimd:  casting DMAs f32 HBM -> bf16 SBUF for pred and target
  VectorE: u = 1-2t (4x); pair sums vb = u + shift(u) (2x, odd tiles
           only); s = p*u (2x); one fused (Su' != 0)*loss pass straight
           from PSUM per tile
  ScalarE: g = exp(s); loss = ln(g*1+1) + per-row loss-sum accumulation,
           each fused across a tile group
  TensorE: Su' per 512-half as A@vb(0) + A@vb(3) + A_cen@u(2) on 1-of-4
           tiles, or plain 5-tap shifted matmuls on the rest (trades idle
           TensorE time for the vb pass on the busier VectorE)
Loss sums ride the ln pass's accum_out (per-row, per exp/ln group); groups
pair only tiles with identical owned-row ranges so the host can mask them.
Host: float64 reduction of per-core partials (owned-row masks per tile).
"""

import sys

import numpy as np

sys.path.insert(0, "/opt/trn_rl_repo")

import ml_dtypes

B, H, W = 32, 1024, 1024
NCORES = 8
SPC = B // NCORES  # samples per core
ROWS = SPC * H
N_TOT = float(B * H * W)
WP = W + 4  # padded width for the 5-tap row window

# per-sample tiling: (input_row_start, input_rows, owned_lo, owned_hi)
TILES = [(0, 128, 0, 126)]
for t in range(1, 8):
    TILES.append((124 * t, 128, 2, 126))
TILES.append((992, 32, 2, 32))
NT = len(TILES)  # 9
NTILES = SPC * NT  # 36

BF16 = ml_dtypes.bfloat16


def _band() -> np.ndarray:
    a = np.zeros((128, 128), dtype=np.float32)
    for k in range(128):
        for m in range(128):
            if abs(k - m) <= 2:
                a[k, m] = 1.0
    return a


def _vh_of(tile_idx: int) -> np.ndarray:
    """clipped column-window size per in-tile row."""
    in0, p_in, _, _ = TILES[tile_idx]
    vh = np.full(128, 5.0, dtype=np.float32)
    for k in range(p_in):
        img = in0 + k
        vh[k] = min(img, 2) + min(H - 1 - img, 2) + 1
    return vh


def _owned(tile_idx: int) -> np.ndarray:
    _, _, o0, o1 = TILES[tile_idx]
    m = np.zeros(128, dtype=np.float32)
    m[o0:o1] = 1.0
    return m


def _statics() -> dict[str, np.ndarray]:
    s = {"a_full": _band().astype(BF16)}
    for nm, ti in (("top", 0), ("mid", 1), ("last", 8)):
        vh = _vh_of(ti)
        s[f"own_{nm}"] = _owned(ti).astype(BF16).reshape(128, 1)
        # center-tap matrix: band - 5*vh[row] on the diagonal (all values
        # small integers, exact in bf16)
        ac = _band()
        for k in range(128):
            ac[k, k] -= 5.0 * vh[k]
        s[f"a_cen_{nm}"] = ac.astype(BF16)
    return s


_CACHED = {}


def _split_multi_waits(nc, mybir):
    """This walrus's core_v3 codegen allows only one sem-wait per
    instruction; peel extra waits onto same-engine NOPs placed just before."""
    skip = (mybir.InstEventSemaphore,)
    k = 0
    for fn in nc.m.functions:
        for blk in fn.blocks:
            out = []
            for inst in blk.instructions:
                si = inst.sync_info
                if (si is not None and len(si.on_wait) > 1
                        and not isinstance(inst, skip)):
                    waits = list(si.on_wait)
                    for w in waits[:-1]:
                        k += 1
                        nop = mybir.InstNoOp(name=f"wsplit-{k}", ins=[], outs=[])
                        nop.engine = inst.engine
                        nop.sync_info = mybir.SyncInfo(on_wait=[w], on_update=[])
                        out.append(nop)
                    inst.sync_info = mybir.SyncInfo(
                        on_wait=[waits[-1]], on_update=list(si.on_update))
                out.append(inst)
            blk.instructions = out


def _build_nc():
    import concourse.bass as bass
    import concourse.mybir as mybir
    import concourse.tile as tile

    f32 = mybir.dt.float32
    bf16 = mybir.dt.bfloat16
    Act = mybir.ActivationFunctionType
    Alu = mybir.AluOpType

    nc = bass.Bass("TRN2", target_bir_lowering=False, debug=False,
                   num_devices=NCORES, num_swdge_queues=4)

    pred_d = nc.dram_tensor("pred", [ROWS, W], f32, kind="ExternalInput").ap()
    tgt_d = nc.dram_tensor("target", [ROWS, W], f32, kind="ExternalInput").ap()
    sd = {}
    statics = _statics()
    for nm, arr in statics.items():
        sd[nm] = nc.dram_tensor(nm, list(arr.shape), bf16,
                                kind="ExternalInput").ap()
    NG = 6 * SPC  # loss-sum accumulation groups (see PAIRS)
    oedge_d = nc.dram_tensor("out_edge", [128, NTILES + NG], f32,
                             kind="ExternalOutput").ap()

    with tile.TileContext(nc) as tc:
        with (
            tc.tile_pool(name="sing", bufs=1) as sing,
            tc.tile_pool(name="tb", bufs=10) as tb_pool,
            tc.tile_pool(name="pb", bufs=10) as pb_pool,
            tc.tile_pool(name="vb", bufs=6) as vb_pool,
            tc.tile_pool(name="s", bufs=4) as s_pool,
            tc.tile_pool(name="g", bufs=4) as g_pool,
            tc.tile_pool(name="loss", bufs=5) as loss_pool,
            tc.tile_pool(name="scr", bufs=6) as scr_pool,
            tc.tile_pool(name="psum", bufs=4, space="PSUM") as psum_pool,
        ):
            # ---- statics in SBUF ----
            sb = {}
            for nm, arr in statics.items():
                sb[nm] = sing.tile(list(arr.shape), bf16, tag=nm, name=nm)
                nc.sync.dma_start(out=sb[nm][:], in_=sd[nm][:])

            def per_tile(t):
                nm = "top" if t == 0 else ("last" if t == NT - 1 else "mid")
                return sb[f"own_{nm}"], sb[f"a_cen_{nm}"]

            stats = sing.tile([128, NTILES + NG], f32, tag="stats")
            nc.vector.memset(stats[:], 0.0)

            # padded ring buffers (pads zeroed once, never rewritten)
            u_bufs = [sing.tile([128, WP], bf16, tag=f"ub{i}",
                                name=f"ub{i}") for i in range(6)]
            for bb in u_bufs:
                nc.vector.memset(bb[:, 0:2], 0.0)
                nc.vector.memset(bb[:, W + 2:WP], 0.0)

            a_sb = sb["a_full"]
            pending = []  # (idx, p_in, sup, loss2, blk)

            def stage_b():
                # Edge-masked loss sums for a tile whose Su'/loss are 2+
                # tiles old, so no engine queue head ever stalls on a
                # fresh cross-engine dependency.  One fused STT does the
                # whole edge test: (Su' != 0)*loss straight from PSUM
                # (exact integer test), accumulated per partition.  Host
                # masks rows to owned per tile.
                bidx, p_in, sup, loss2, blk = pending.pop(0)
                lo = blk * W
                scr = scr_pool.tile([128, W], bf16)
                nc.vector.scalar_tensor_tensor(
                    out=scr[0:p_in], in0=sup[0:p_in],
                    scalar=0.0, in1=loss2[0:p_in, lo:lo + W],
                    op0=Alu.not_equal, op1=Alu.mult,
                    accum_out=stats[0:p_in, bidx:bidx + 1])
                if bidx == 17:
                    # first half of the stats early, off the tail
                    nc.sync.dma_start(out=oedge_d[:, 0:18],
                                      in_=stats[:, 0:18])

            # DVE/DMA work stays per-tile (fine-grained pipelining); only
            # exp/ln are fused per tile-PAIR through a shared s2 buffer,
            # halving ScalarE's fixed per-instruction cost.
            # the 32-row tile goes FIRST: its small DMAs land ~4x
            # sooner, so the pipeline's first compute chain starts while
            # tile 0's full-size DMA is still streaming
            PAIRS = [(8,), (0,), (1, 2), (3, 4), (5, 6), (7,)]
            idx = 0
            uidx = 0
            aidx = 0
            for smp in range(SPC):
                for gi, ts in enumerate(PAIRS):
                    nblk = len(ts)
                    P = max(TILES[t][1] for t in ts)
                    s2 = s_pool.tile([128, 2048], bf16)
                    pair = []
                    for b, t in enumerate(ts):
                        in0, p_in, o0, o1 = TILES[t]
                        r0 = smp * H + in0

                        # casting DMAs: f32 HBM -> bf16 SBUF
                        tb = tb_pool.tile([128, W], bf16)
                        nc.gpsimd.dma_start(out=tb[0:p_in],
                                            in_=tgt_d[r0:r0 + p_in, :])
                        pb = pb_pool.tile([128, W], bf16)
                        nc.gpsimd.dma_start(out=pb[0:p_in],
                                            in_=pred_d[r0:r0 + p_in, :])

                        # u = 1 - 2t into padded buffer center (4x mode)
                        ub = u_bufs[uidx % 6]
                        uidx += 1
                        nc.vector.tensor_scalar(
                            out=ub[0:p_in, 2:2 + W], in0=tb[0:p_in],
                            scalar1=-2.0, scalar2=1.0, op0=Alu.mult,
                            op1=Alu.add)

                        # pair sums: vb[j] = u[j] + u[j+1]  (2x mode).
                        # Only every other tile: the even tiles run plain
                        # 5-tap matmuls instead, trading idle TensorE
                        # time for DVE time (DVE is the pacer).
                        if aidx % 4 == 3:
                            vb = vb_pool.tile([128, WP], bf16)
                            nc.vector.tensor_add(out=vb[0:p_in, 0:W + 3],
                                                 in0=ub[0:p_in, 0:W + 3],
                                                 in1=ub[0:p_in, 1:W + 4])
                        else:
                            vb = None

                        # s = p*u (all bf16, 2x) into this pair's block
                        nc.vector.tensor_mul(
                            out=s2[0:p_in, b * W:b * W + W],
                            in0=pb[0:p_in],
                            in1=ub[0:p_in, 2:2 + W])
                        pair.append((t, b, ub, vb))
                        aidx += 1

                    # loss = ln(exp(s)+1), the whole group in one ACT
                    # pass; ln accumulates per-row loss sums (host masks
                    # rows to the group's shared owned range)
                    g2 = g_pool.tile([128, 2048], bf16)
                    nc.scalar.activation(out=g2[0:P, 0:nblk * W],
                                         in_=s2[0:P, 0:nblk * W],
                                         func=Act.Exp)
                    gcol = NTILES + smp * len(PAIRS) + gi
                    loss2 = loss_pool.tile([128, 2048], bf16)
                    nc.scalar.activation(out=loss2[0:P, 0:nblk * W],
                                         in_=g2[0:P, 0:nblk * W],
                                         func=Act.Ln, bias=1.0,
                                         accum_out=stats[0:P,
                                                         gcol:gcol + 1])

                    # Su': either A@vb(0) + A@vb(3) + A_cen@u(2) (3-tap,
                    # vb tiles) or plain 5-tap shifted matmuls on u
                    for t, b, ub, vb in pair:
                        p_in = TILES[t][1]
                        _, acen_sb = per_tile(t)
                        sup = psum_pool.tile([128, W], f32)
                        for h in (0, 512):
                            if vb is not None:
                                nc.tensor.matmul(
                                    sup[:, h:h + 512], a_sb[0:p_in, :],
                                    vb[0:p_in, h:h + 512],
                                    start=True, stop=False)
                                nc.tensor.matmul(
                                    sup[:, h:h + 512], a_sb[0:p_in, :],
                                    vb[0:p_in, h + 3:h + 515],
                                    start=False, stop=False)
                            else:
                                for dd, first in ((0, True), (1, False),
                                                  (3, False), (4, False)):
                                    nc.tensor.matmul(
                                        sup[:, h:h + 512], a_sb[0:p_in, :],
                                        ub[0:p_in, h + dd:h + dd + 512],
                                        start=first, stop=False)
                            nc.tensor.matmul(
                                sup[:, h:h + 512], acen_sb[0:p_in, :],
                                ub[0:p_in, h + 2:h + 514],
                                start=False, stop=True)
                        pending.append((idx, p_in, sup, loss2, b))
                        idx += 1
                    # keep a 2-tile skew mid-run for decoupling, but
                    # drain eagerly near the end so the final STTs
                    # overlap the last groups' exp/ln/matmul work
                    lim = 1 if (smp < SPC - 1 or gi < 4) else 0
                    while len(pending) > lim:
                        stage_b()
            while pending:
                stage_b()

            nc.sync.dma_start(out=oedge_d[:, 18:NTILES + NG],
                              in_=stats[:, 18:NTILES + NG])

    _split_multi_waits(nc, mybir)
    return nc


def _get_nc():
    if "nc" not in _CACHED:
        _CACHED["nc"] = _build_nc()
    return _CACHED["nc"]


def run(pred: np.ndarray, target: np.ndarray, trace: bool = False):
    """Returns (result_scalar, BassKernelResults)."""
    from concourse import bass_utils

    nc = _get_nc()
    statics = _statics()
    pred = np.ascontiguousarray(np.asarray(pred).reshape(B * H, W),
                                dtype=np.float32)
    target = np.ascontiguousarray(np.asarray(target).reshape(B * H, W),
                                  dtype=np.float32)
    in_maps = []
    for c in range(NCORES):
        m = dict(statics)
        m["pred"] = pred[c * ROWS:(c + 1) * ROWS]
        m["target"] = target[c * ROWS:(c + 1) * ROWS]
        in_maps.append(m)
    res = bass_utils.run_bass_kernel_spmd(
        nc, in_maps, core_ids=list(range(NCORES)), trace=trace)
    # loss-sum group -> shared owned-row range (matches PAIRS in build)
    GROUPS = [(8,), (0,), (1, 2), (3, 4), (5, 6), (7,)]
    s_loss = 0.0
    s_le = 0.0
    for r in res.results:
        e = r["out_edge"].astype(np.float64)
        order = [8, 0, 1, 2, 3, 4, 5, 6, 7]  # per-sample processing order
        for ti in range(NTILES):
            _, _, o0, o1 = TILES[order[ti % NT]]
            s_le += e[o0:o1, ti].sum()
        for smp in range(SPC):
            for gi, ts in enumerate(GROUPS):
                _, _, o0, o1 = TILES[ts[0]]
                gcol = NTILES + smp * len(GROUPS) + gi
                s_loss += e[o0:o1, gcol].sum()
    val = np.float32((s_loss - 0.9 * s_le) / N_TOT)
    return np.asarray(val, dtype=np.float32), res


def kernel(pred: np.ndarray, target: np.ndarray) -> np.ndarray:
    val, _ = run(pred, target, trace=False)
    return val


if __name__ == "__main__":
    rng = np.random.default_rng(0)
    p = rng.standard_normal((B, 1, H, W)).astype(np.float32)
    t = rng.integers(0, 2, (B, 1, H, W)).astype(np.float32)
    print(kernel(pred=p, target=t))
